# revision 2
# baseline (speedup 1.0000x reference)
"""AdjacencyMatchingLoss on 8 trn2 NeuronCores — self-contained.

Math (per batch b):
    A[p,q] = (d_hw[p,q] == 1)
    loss   = -mean_b( sum_e w_be * (P_b A)[src_be] . P_b[dst_be] / max(sum_e w_be, eps) )

All-matmul fp8 DoubleRow rewrite ("R-form", no transposes on device):
    wt_be      = 512 * w_be / max(sum_e w_be, eps)          (fp8 per-edge)
    Wt[j,i]    = sum_e [dst_be==j] [src_be==i] wt_be        (one-hot DR matmuls)
    Ut[q,i]    = sum_j P_b[j,q] Wt[j,i]                     (DR matmul, = 512*U^T)
    R[p,i]     = sum_q A[p,q] Ut[q,i]                       (DR matmul vs A^T from host)
    partial_b  = sum_{p,i} R[p,i] P_b[i,p] / 512            (small DVE mult + ACT accum)
    loss       = -(1/B) sum_cores sum_b partial_b

Host ships fp8 casts/layouts only: P8 [b,i,q], Pt8 = P^T [b,p,i], At8 = A^T [q,p].
Data-parallel over batch: 2 batches/core, host sums 8 scalars.
"""

import numpy as np

B, NLOG, NPHYS, E = 16, 512, 2048, 2048
NCORES = 8
BLOC = B // NCORES          # batches per core
NI = NLOG // 128            # 4  i/j-chunks per batch
NE = E // 128               # 16 e-chunks per batch
NEP = NE // 2               # 8  e-pair chunks (DoubleRow)
NP = NPHYS // 128           # 16 p/q-chunks
KV = BLOC * NI              # 8  j-chunks stacked over batches (P8 layout)
EPS = 1e-8
DR = None                   # set in _emit

_CACHE = {}


def _emit(tc, aps):
    from contextlib import ExitStack

    from concourse import mybir

    nc = tc.nc
    f32 = mybir.dt.float32
    f16 = mybir.dt.float16
    f8 = mybir.dt.float8e4
    i32 = mybir.dt.int32
    AO = mybir.AluOpType
    ACT_COPY = mybir.ActivationFunctionType.Copy
    DRM = mybir.MatmulPerfMode.DoubleRow

    P8_ap = aps["P8"]
    Pt8_ap = aps["Pt8"]
    At8_ap = aps["At8"]
    src_ap = aps["edge_src"]
    dst_ap = aps["edge_dst"]
    w_ap = aps["edge_w"]
    iota_ap = aps["iota"]      # [128, 512] f16: row 0..511 on every partition
    ones_ap = aps["ones"]      # [128, 128] f32: all ones
    out_ap = aps["out"]

    ctx = ExitStack()
    with ctx:
        const = ctx.enter_context(tc.tile_pool(name="const", bufs=1))
        big = ctx.enter_context(tc.tile_pool(name="big", bufs=1))
        wtp = ctx.enter_context(tc.tile_pool(name="wtp", bufs=2))
        ohp = ctx.enter_context(tc.tile_pool(
            name="ohp", bufs=aps.get("_ohp_bufs", 4)))
        edg = ctx.enter_context(tc.tile_pool(name="edg", bufs=2))
        accp = ctx.enter_context(tc.tile_pool(name="accp", bufs=3))
        scr = ctx.enter_context(tc.tile_pool(
            name="scr", bufs=aps.get("_scr_bufs", 3)))
        psum = ctx.enter_context(tc.tile_pool(name="psum", bufs=2, space="PSUM"))

        def _body():
            # host-provided constants
            iota16 = const.tile([128, 512], f16)
            nc.sync.dma_start(iota16, iota_ap)
            ones = const.tile([128, 128], f32)
            nc.sync.dma_start(ones, ones_ap)

            # ---- bulk input DMA ----
            P8 = big.tile([128, KV, NPHYS], f8, tag="P8")
            for b in range(BLOC):
                for ic in range(NI):
                    nc.sync.dma_start(
                        P8[:, b * NI + ic, :],
                        P8_ap[b, ic * 128:(ic + 1) * 128, :],
                    )
            At = big.tile([128, NP, NPHYS], f8, tag="At")
            for qc in range(NP):
                nc.sync.dma_start(
                    At[:, qc, :], At8_ap[qc * 128:(qc + 1) * 128, :]
                )
            Pt = big.tile([128, BLOC * NP, 512], f8, tag="Pt")
            for b in range(BLOC):
                for pc in range(NP):
                    nc.sync.dma_start(
                        Pt[:, b * NP + pc, :],
                        Pt8_ap[b, pc * 128:(pc + 1) * 128, :],
                    )
            Ut8 = big.tile([128, BLOC * NP, 512], f8, tag="Ut8")

            # ---- per-batch: edges -> one-hots -> Wt -> Ut ----
            for b in range(BLOC):
                src32 = edg.tile([128, NE], i32, tag="src32")
                nc.sync.dma_start(src32, src_ap[b].rearrange("(p c) -> p c", p=128))
                dst32 = edg.tile([128, NE], i32, tag="dst32")
                nc.sync.dma_start(dst32, dst_ap[b].rearrange("(p c) -> p c", p=128))
                wf = edg.tile([128, NE], f32, tag="wf")
                nc.sync.dma_start(wf, w_ap[b].rearrange("(p c) -> p c", p=128))

                srcf = edg.tile([128, NE], f32, tag="srcf")
                nc.vector.tensor_copy(srcf, src32)
                dstf = edg.tile([128, NE], f32, tag="dstf")
                nc.vector.tensor_copy(dstf, dst32)

                # wt512 = 512 * w / max(sum(w), eps), broadcast via ones-matmul
                swp = edg.tile([128, 1], f32, tag="swp")
                nc.vector.tensor_reduce(swp, wf, axis=mybir.AxisListType.X, op=AO.add)
                ps_sw = psum.tile([128, 2048], f32, tag="ps")
                nc.tensor.matmul(ps_sw[:, 0:1], ones, swp)
                swm = edg.tile([128, 1], f32, tag="swm")
                nc.vector.tensor_scalar_max(swm, ps_sw[:, 0:1], EPS)
                rsw = edg.tile([128, 1], f32, tag="rsw")
                nc.vector.reciprocal(rsw, swm)
                wtf = edg.tile([128, NE], f32, tag="wtf")
                nc.vector.tensor_scalar(wtf, wf, rsw, 512.0, op0=AO.mult, op1=AO.mult)

                # Wt[j,i] via DoubleRow one-hot matmuls; 4 j-banks in one psum tile
                ps_wt = psum.tile([128, 2048], f32, tag="ps")
                for cp in range(NEP):
                    od2 = ohp.tile([128, 2, 512], f8, tag="od2")
                    osw2 = ohp.tile([128, 2, 512], f8, tag="osw2")
                    for h in (0, 1):
                        c = 2 * cp + h
                        nc.vector.tensor_scalar(
                            od2[:, h, :], iota16, dstf[:, c:c + 1], None,
                            op0=AO.is_equal,
                        )
                        nc.vector.tensor_scalar(
                            osw2[:, h, :], iota16, srcf[:, c:c + 1], wtf[:, c:c + 1],
                            op0=AO.is_equal, op1=AO.mult,
                        )
                    for j in range(NI):
                        nc.tensor.matmul(
                            ps_wt[:, j * 512:(j + 1) * 512],
                            od2[:, :, j * 128:(j + 1) * 128],
                            osw2,
                            start=(cp == 0),
                            stop=(cp == NEP - 1),
                            perf_mode=DRM,
                        )
                wt8 = wtp.tile([128, NI, 512], f8, tag="wt8")
                nc.scalar.copy(wt8, ps_wt.rearrange("p (j i) -> p j i", j=NI))

                # Ut[q,i] = sum_j P[j,q] Wt[j,i]; q-tiles in groups of 4
                for qg in range(NP // 4):
                    ps_ut = psum.tile([128, 2048], f32, tag="ps")
                    for q4 in range(4):
                        qc = qg * 4 + q4
                        for g in range(NI // 2):
                            nc.tensor.matmul(
                                ps_ut[:, q4 * 512:(q4 + 1) * 512],
                                P8[:, b * NI + 2 * g:b * NI + 2 * g + 2,
                                   qc * 128:(qc + 1) * 128],
                                wt8[:, 2 * g:2 * g + 2, :],
                                start=(g == 0),
                                stop=(g == NI // 2 - 1),
                                perf_mode=DRM,
                            )
                    nc.scalar.copy(
                        Ut8[:, b * NP + qg * 4:b * NP + qg * 4 + 4, :],
                        ps_ut.rearrange("p (c i) -> p c i", c=4),
                    )

            # ---- R = A Ut, consumed against P^T; p-tiles in groups of 4 ----
            acc = None
            for b in range(BLOC):
                for pg in range(NP // 4):
                    ps_r = psum.tile([128, 2048], f32, tag="ps")
                    for p4 in range(4):
                        pc = pg * 4 + p4
                        for k in range(NP // 2):
                            nc.tensor.matmul(
                                ps_r[:, p4 * 512:(p4 + 1) * 512],
                                At[:, 2 * k:2 * k + 2, pc * 128:(pc + 1) * 128],
                                Ut8[:, b * NP + 2 * k:b * NP + 2 * k + 2, :],
                                start=(k == 0),
                                stop=(k == NP // 2 - 1),
                                perf_mode=DRM,
                            )
                    tmp = scr.tile([128, 2048], f16, tag="tmp")
                    nc.vector.tensor_tensor(
                        tmp,
                        ps_r,
                        Pt[:, b * NP + pg * 4:b * NP + pg * 4 + 4, :].rearrange(
                            "p c i -> p (c i)"),
                        AO.mult,
                    )
                    pacc = accp.tile([128, 1], f32, tag="pacc")
                    trash = scr.tile([128, 2048], f16, tag="trash")
                    nc.scalar.activation(trash, tmp, ACT_COPY, accum_out=pacc)
                    if acc is None:
                        acc = pacc
                    else:
                        nacc = accp.tile([128, 1], f32, tag="acc")
                        nc.scalar.add(nacc, pacc, acc[:, 0:1])
                        acc = nacc

            # ---- partition-reduce via ones-matmul broadcast, write out ----
            ps_f = psum.tile([128, 2048], f32, tag="ps")
            nc.tensor.matmul(ps_f[:, 0:1], ones, acc)
            res = const.tile([1, 1], f32)
            nc.scalar.mul(res, ps_f[0:1, 0:1], 1.0 / 512.0)
            nc.sync.dma_start(out_ap, res)

        if "_nrep" in aps:
            nrt = const.tile([1, 1], mybir.dt.int32)
            nc.sync.dma_start(nrt, aps["_nrep"])
            nval = nc.values_load(nrt[0:1, 0:1], min_val=1, max_val=4096,
                                  skip_runtime_bounds_check=True)
            with tc.For_i(0, nval, 1):
                _body()
        else:
            _body()


def build(repeat=1, loop_rt=False, probe_flags=()):
    import concourse.tile as tile
    from concourse import bacc, mybir

    f32 = mybir.dt.float32
    f8 = mybir.dt.float8e4
    i32 = mybir.dt.int32
    nc = bacc.Bacc(
        "TRN2", target_bir_lowering=False, debug=False, num_devices=NCORES
    )
    aps = {
        "P8": nc.dram_tensor("P8", [BLOC, NLOG, NPHYS], f8, kind="ExternalInput").ap(),
        "Pt8": nc.dram_tensor("Pt8", [BLOC, NPHYS, NLOG], f8, kind="ExternalInput").ap(),
        "At8": nc.dram_tensor("At8", [NPHYS, NPHYS], f8, kind="ExternalInput").ap(),
        "edge_src": nc.dram_tensor("edge_src", [BLOC, E], i32, kind="ExternalInput").ap(),
        "edge_dst": nc.dram_tensor("edge_dst", [BLOC, E], i32, kind="ExternalInput").ap(),
        "edge_w": nc.dram_tensor("edge_w", [BLOC, E], f32, kind="ExternalInput").ap(),
        "iota": nc.dram_tensor("iota", [128, 512], mybir.dt.float16, kind="ExternalInput").ap(),
        "ones": nc.dram_tensor("ones", [128, 128], f32, kind="ExternalInput").ap(),
        "out": nc.dram_tensor("out", [1, 1], f32, kind="ExternalOutput").ap(),
    }
    for fl in probe_flags:
        if isinstance(fl, tuple):
            aps[fl[0]] = fl[1]
        else:
            aps[fl] = True
    if loop_rt:
        aps["_nrep"] = nc.dram_tensor("nrep", [1, 1], i32, kind="ExternalInput").ap()
    with tile.TileContext(nc) as tc:
        for _ in range(repeat):
            _emit(tc, aps)
    nc.compile()
    return nc


def shard_inputs(P, d_hw, edge_src, edge_dst, edge_w):
    import ml_dtypes

    f8 = ml_dtypes.float8_e4m3fn
    P = np.asarray(P, dtype=np.float32)
    P8 = np.ascontiguousarray(P.astype(f8))
    Pt8 = np.ascontiguousarray(P.transpose(0, 2, 1)).astype(f8)
    At8 = np.ascontiguousarray(
        (np.asarray(d_hw) == 1).T.astype(f8))
    edge_src = np.ascontiguousarray(np.asarray(edge_src, dtype=np.int32))
    edge_dst = np.ascontiguousarray(np.asarray(edge_dst, dtype=np.int32))
    edge_w = np.ascontiguousarray(np.asarray(edge_w, dtype=np.float32))
    iota = np.broadcast_to(np.arange(512, dtype=np.float16), (128, 512)).copy()
    ones = np.ones((128, 128), dtype=np.float32)
    in_maps = []
    for c in range(NCORES):
        sl = slice(c * BLOC, (c + 1) * BLOC)
        in_maps.append(
            {
                "P8": P8[sl],
                "Pt8": Pt8[sl],
                "At8": At8,
                "edge_src": edge_src[sl],
                "edge_dst": edge_dst[sl],
                "edge_w": edge_w[sl],
                "iota": iota,
                "ones": ones,
            }
        )
    return in_maps


def kernel(P, d_hw, edge_src, edge_dst, edge_w):
    from concourse.bass_utils import run_bass_kernel_spmd

    if "nc" not in _CACHE:
        _CACHE["nc"] = build()
    nc = _CACHE["nc"]
    in_maps = shard_inputs(P, d_hw, edge_src, edge_dst, edge_w)
    res = run_bass_kernel_spmd(nc, in_maps, core_ids=list(range(NCORES)))
    partial = sum(float(res.results[c]["out"][0, 0]) for c in range(NCORES))
    return np.float32(-partial / B)


# revision 3
# speedup vs baseline: 1.1861x; 1.1861x over previous
"""AdjacencyMatchingLoss on 8 trn2 NeuronCores — self-contained.

Math (per batch b):
    A[p,q] = (d_hw[p,q] == 1)
    loss   = -mean_b( sum_e w_be * (P_b A)[src_be] . P_b[dst_be] / max(sum_e w_be, eps) )

All-matmul fp8 DoubleRow rewrite ("R-form", no transposes on device):
    wt_be      = 512 * w_be / max(sum_e w_be, eps)          (fp8 per-edge)
    Wt[j,i]    = sum_e [dst_be==j] [src_be==i] wt_be        (one-hot DR matmuls)
    Ut[q,i]    = sum_j P_b[j,q] Wt[j,i]                     (DR matmul, = 512*U^T)
    R[p,i]     = sum_q A[p,q] Ut[q,i]                       (DR matmul vs A^T from host)
    partial_b  = sum_{p,i} R[p,i] P_b[i,p] / 512            (small DVE mult + ACT accum)
    loss       = -(1/B) sum_cores sum_b partial_b

Host ships fp8 casts/layouts only: P8 [b,i,q], Pt8 = P^T [b,p,i], At8 = A^T [q,p].
Data-parallel over batch: 2 batches/core, host sums 8 scalars.
"""

import numpy as np

B, NLOG, NPHYS, E = 16, 512, 2048, 2048
NCORES = 8
BLOC = B // NCORES          # batches per core
NI = NLOG // 128            # 4  i/j-chunks per batch
NE = E // 128               # 16 e-chunks per batch
NEP = NE // 2               # 8  e-pair chunks (DoubleRow)
NP = NPHYS // 128           # 16 p/q-chunks
KV = BLOC * NI              # 8  j-chunks stacked over batches (P8 layout)
EPS = 1e-8
DR = None                   # set in _emit

_CACHE = {}


def _emit(tc, aps):
    from contextlib import ExitStack

    from concourse import mybir

    nc = tc.nc
    f32 = mybir.dt.float32
    f16 = mybir.dt.float16
    f8 = mybir.dt.float8e4
    i32 = mybir.dt.int32
    AO = mybir.AluOpType
    ACT_COPY = mybir.ActivationFunctionType.Copy
    DRM = mybir.MatmulPerfMode.DoubleRow

    P8_ap = aps["P8"]
    Pt8_ap = aps["Pt8"]
    At8_ap = aps["At8"]
    src_ap = aps["edge_src"]
    dst_ap = aps["edge_dst"]
    w_ap = aps["edge_w"]
    iota_ap = aps["iota"]      # [128, 512] f16: row 0..511 on every partition
    ones_ap = aps["ones"]      # [128, 128] f32: all ones
    out_ap = aps["out"]

    ctx = ExitStack()
    with ctx:
        const = ctx.enter_context(tc.tile_pool(name="const", bufs=1))
        big = ctx.enter_context(tc.tile_pool(name="big", bufs=1))
        wtp = ctx.enter_context(tc.tile_pool(name="wtp", bufs=2))
        ohp = ctx.enter_context(tc.tile_pool(
            name="ohp", bufs=aps.get("_ohp_bufs", 4)))
        edg = ctx.enter_context(tc.tile_pool(name="edg", bufs=2))
        accp = ctx.enter_context(tc.tile_pool(name="accp", bufs=3))
        scr = ctx.enter_context(tc.tile_pool(
            name="scr", bufs=aps.get("_scr_bufs", 3)))
        psum = ctx.enter_context(tc.tile_pool(name="psum", bufs=2, space="PSUM"))

        def _body():
            # host-provided constants
            iota16 = const.tile([128, 512], f16)
            nc.sync.dma_start(iota16, iota_ap)
            ones = const.tile([128, 128], f32)
            nc.sync.dma_start(ones, ones_ap)

            # ---- bulk input DMA: few big strided descriptors, spread over
            # the three DMA-capable queues (sync / scalar / gpsimd) ----
            src32 = edg.tile([128, BLOC, NE], i32, tag="src32")
            nc.sync.dma_start(src32, src_ap.rearrange("b (p c) -> p b c", p=128))
            dst32 = edg.tile([128, BLOC, NE], i32, tag="dst32")
            nc.sync.dma_start(dst32, dst_ap.rearrange("b (p c) -> p b c", p=128))
            wf2 = edg.tile([128, BLOC, NE], f32, tag="wf")
            nc.sync.dma_start(wf2, w_ap.rearrange("b (p c) -> p b c", p=128))

            P8 = big.tile([128, KV, NPHYS], f8, tag="P8")
            for b in range(BLOC):
                nc.sync.dma_start(
                    P8[:, b * NI:(b + 1) * NI, :],
                    P8_ap[b].rearrange("(k p) q -> p k q", p=128),
                )
            Pt = big.tile([128, BLOC * NP, 512], f8, tag="Pt")
            for b in range(BLOC):
                nc.sync.dma_start(
                    Pt[:, b * NP:(b + 1) * NP, :],
                    Pt8_ap[b].rearrange("(k p) i -> p k i", p=128),
                )
            At = big.tile([128, NP, NPHYS], f8, tag="At")
            At_src = At8_ap.rearrange("(k p) q -> p k q", p=128)
            nc.gpsimd.dma_start(At[:, 0:NP // 2, :], At_src[:, 0:NP // 2, :])
            nc.scalar.dma_start(At[:, NP // 2:NP, :], At_src[:, NP // 2:NP, :])
            Ut8 = big.tile([128, BLOC * NP, 512], f8, tag="Ut8")

            # ---- per-batch: edges -> one-hots -> Wt -> Ut ----
            for b in range(BLOC):
                srcf = edg.tile([128, NE], f32, tag="srcf")
                nc.vector.tensor_copy(srcf, src32[:, b, :])
                dstf = edg.tile([128, NE], f32, tag="dstf")
                nc.vector.tensor_copy(dstf, dst32[:, b, :])
                wf = wf2[:, b, :]

                # wt512 = 512 * w / max(sum(w), eps), broadcast via ones-matmul
                swp = edg.tile([128, 1], f32, tag="swp")
                nc.vector.tensor_reduce(swp, wf, axis=mybir.AxisListType.X, op=AO.add)
                ps_sw = psum.tile([128, 2048], f32, tag="ps")
                nc.tensor.matmul(ps_sw[:, 0:1], ones, swp)
                swm = edg.tile([128, 1], f32, tag="swm")
                nc.vector.tensor_scalar_max(swm, ps_sw[:, 0:1], EPS)
                rsw = edg.tile([128, 1], f32, tag="rsw")
                nc.vector.reciprocal(rsw, swm)
                wtf = edg.tile([128, NE], f32, tag="wtf")
                nc.vector.tensor_scalar(wtf, wf, rsw, 512.0, op0=AO.mult, op1=AO.mult)

                # Wt[j,i] via DoubleRow one-hot matmuls; 4 j-banks in one psum tile
                ps_wt = psum.tile([128, 2048], f32, tag="ps")
                for cp in range(NEP):
                    od2 = ohp.tile([128, 2, 512], f8, tag="od2")
                    osw2 = ohp.tile([128, 2, 512], f8, tag="osw2")
                    for h in (0, 1):
                        c = 2 * cp + h
                        nc.vector.tensor_scalar(
                            od2[:, h, :], iota16, dstf[:, c:c + 1], None,
                            op0=AO.is_equal,
                        )
                        nc.vector.tensor_scalar(
                            osw2[:, h, :], iota16, srcf[:, c:c + 1], wtf[:, c:c + 1],
                            op0=AO.is_equal, op1=AO.mult,
                        )
                    for j in range(NI):
                        nc.tensor.matmul(
                            ps_wt[:, j * 512:(j + 1) * 512],
                            od2[:, :, j * 128:(j + 1) * 128],
                            osw2,
                            start=(cp == 0),
                            stop=(cp == NEP - 1),
                            perf_mode=DRM,
                        )
                wt8 = wtp.tile([128, NI, 512], f8, tag="wt8")
                nc.scalar.copy(wt8, ps_wt.rearrange("p (j i) -> p j i", j=NI))

                # Ut[q,i] = sum_j P[j,q] Wt[j,i]; q-tiles in groups of 4
                for qg in range(NP // 4):
                    ps_ut = psum.tile([128, 2048], f32, tag="ps")
                    for q4 in range(4):
                        qc = qg * 4 + q4
                        for g in range(NI // 2):
                            nc.tensor.matmul(
                                ps_ut[:, q4 * 512:(q4 + 1) * 512],
                                P8[:, b * NI + 2 * g:b * NI + 2 * g + 2,
                                   qc * 128:(qc + 1) * 128],
                                wt8[:, 2 * g:2 * g + 2, :],
                                start=(g == 0),
                                stop=(g == NI // 2 - 1),
                                perf_mode=DRM,
                            )
                    nc.scalar.copy(
                        Ut8[:, b * NP + qg * 4:b * NP + qg * 4 + 4, :],
                        ps_ut.rearrange("p (c i) -> p c i", c=4),
                    )

            # ---- R = A Ut, consumed against P^T; p-tiles in groups of 4 ----
            acc = None
            for b in range(BLOC):
                for pg in range(NP // 4):
                    ps_r = psum.tile([128, 2048], f32, tag="ps")
                    for p4 in range(4):
                        pc = pg * 4 + p4
                        for k in range(NP // 2):
                            nc.tensor.matmul(
                                ps_r[:, p4 * 512:(p4 + 1) * 512],
                                At[:, 2 * k:2 * k + 2, pc * 128:(pc + 1) * 128],
                                Ut8[:, b * NP + 2 * k:b * NP + 2 * k + 2, :],
                                start=(k == 0),
                                stop=(k == NP // 2 - 1),
                                perf_mode=DRM,
                            )
                    tmp = scr.tile([128, 2048], f16, tag="tmp")
                    nc.vector.tensor_tensor(
                        tmp,
                        ps_r,
                        Pt[:, b * NP + pg * 4:b * NP + pg * 4 + 4, :].rearrange(
                            "p c i -> p (c i)"),
                        AO.mult,
                    )
                    pacc = accp.tile([128, 1], f32, tag="pacc")
                    trash = scr.tile([128, 2048], f16, tag="trash")
                    nc.scalar.activation(trash, tmp, ACT_COPY, accum_out=pacc)
                    if acc is None:
                        acc = pacc
                    else:
                        nacc = accp.tile([128, 1], f32, tag="acc")
                        nc.scalar.add(nacc, pacc, acc[:, 0:1])
                        acc = nacc

            # ---- partition-reduce via ones-matmul broadcast, write out ----
            ps_f = psum.tile([128, 2048], f32, tag="ps")
            nc.tensor.matmul(ps_f[:, 0:1], ones, acc)
            res = const.tile([1, 1], f32)
            nc.scalar.mul(res, ps_f[0:1, 0:1], 1.0 / 512.0)
            nc.sync.dma_start(out_ap, res)

        if "_nrep" in aps:
            nrt = const.tile([1, 1], mybir.dt.int32)
            nc.sync.dma_start(nrt, aps["_nrep"])
            nval = nc.values_load(nrt[0:1, 0:1], min_val=1, max_val=4096,
                                  skip_runtime_bounds_check=True)
            with tc.For_i(0, nval, 1):
                _body()
        else:
            _body()


def build(repeat=1, loop_rt=False, probe_flags=()):
    import concourse.tile as tile
    from concourse import bacc, mybir

    f32 = mybir.dt.float32
    f8 = mybir.dt.float8e4
    i32 = mybir.dt.int32
    nc = bacc.Bacc(
        "TRN2", target_bir_lowering=False, debug=False, num_devices=NCORES
    )
    aps = {
        "P8": nc.dram_tensor("P8", [BLOC, NLOG, NPHYS], f8, kind="ExternalInput").ap(),
        "Pt8": nc.dram_tensor("Pt8", [BLOC, NPHYS, NLOG], f8, kind="ExternalInput").ap(),
        "At8": nc.dram_tensor("At8", [NPHYS, NPHYS], f8, kind="ExternalInput").ap(),
        "edge_src": nc.dram_tensor("edge_src", [BLOC, E], i32, kind="ExternalInput").ap(),
        "edge_dst": nc.dram_tensor("edge_dst", [BLOC, E], i32, kind="ExternalInput").ap(),
        "edge_w": nc.dram_tensor("edge_w", [BLOC, E], f32, kind="ExternalInput").ap(),
        "iota": nc.dram_tensor("iota", [128, 512], mybir.dt.float16, kind="ExternalInput").ap(),
        "ones": nc.dram_tensor("ones", [128, 128], f32, kind="ExternalInput").ap(),
        "out": nc.dram_tensor("out", [1, 1], f32, kind="ExternalOutput").ap(),
    }
    for fl in probe_flags:
        if isinstance(fl, tuple):
            aps[fl[0]] = fl[1]
        else:
            aps[fl] = True
    if loop_rt:
        aps["_nrep"] = nc.dram_tensor("nrep", [1, 1], i32, kind="ExternalInput").ap()
    with tile.TileContext(nc) as tc:
        for _ in range(repeat):
            _emit(tc, aps)
    nc.compile()
    return nc


def shard_inputs(P, d_hw, edge_src, edge_dst, edge_w):
    import ml_dtypes

    f8 = ml_dtypes.float8_e4m3fn
    P = np.asarray(P, dtype=np.float32)
    P8 = np.ascontiguousarray(P.astype(f8))
    Pt8 = np.ascontiguousarray(P.transpose(0, 2, 1)).astype(f8)
    At8 = np.ascontiguousarray(
        (np.asarray(d_hw) == 1).T.astype(f8))
    edge_src = np.ascontiguousarray(np.asarray(edge_src, dtype=np.int32))
    edge_dst = np.ascontiguousarray(np.asarray(edge_dst, dtype=np.int32))
    edge_w = np.ascontiguousarray(np.asarray(edge_w, dtype=np.float32))
    iota = np.broadcast_to(np.arange(512, dtype=np.float16), (128, 512)).copy()
    ones = np.ones((128, 128), dtype=np.float32)
    in_maps = []
    for c in range(NCORES):
        sl = slice(c * BLOC, (c + 1) * BLOC)
        in_maps.append(
            {
                "P8": P8[sl],
                "Pt8": Pt8[sl],
                "At8": At8,
                "edge_src": edge_src[sl],
                "edge_dst": edge_dst[sl],
                "edge_w": edge_w[sl],
                "iota": iota,
                "ones": ones,
            }
        )
    return in_maps


def kernel(P, d_hw, edge_src, edge_dst, edge_w):
    from concourse.bass_utils import run_bass_kernel_spmd

    if "nc" not in _CACHE:
        _CACHE["nc"] = build()
    nc = _CACHE["nc"]
    in_maps = shard_inputs(P, d_hw, edge_src, edge_dst, edge_w)
    res = run_bass_kernel_spmd(nc, in_maps, core_ids=list(range(NCORES)))
    partial = sum(float(res.results[c]["out"][0, 0]) for c in range(NCORES))
    return np.float32(-partial / B)
